# revision 11
# baseline (speedup 1.0000x reference)
"""Trainium2 Bass kernel for nn_Attention3 (cross-attention, softmax over query axis).

Math (per batch b):
    q = enc[b] @ W_q;  k = dec[b] @ W_k;  v = z[b] @ W_v
    S[q,k] = (q . k) / sqrt(H);  masked rows (mask[b,q]==0) -> -1e9
    attn = softmax over q axis;  out = attn-weighted sum of v

Kernel strategy (data-parallel over batch across 8 cores, 4 batches/core):
  - Host folds W_q/W_k into G = W_k @ W_q^T so the device computes
    S^T = dec @ G @ enc^T with two matmuls instead of three.
  - Everything on-device is computed in the transposed S^T[k, q] layout so the
    softmax reduction over q is a free-axis (per-partition) reduce.
  - Activations are transposed on the PE (identity matmul) since fp32 has no
    DMA-transpose path.
  - Score matmuls run in fp32 PE mode (precision: softmax logits have std ~1e3,
    so bf16-class score matmuls would corrupt the attention hard-max); the
    value/output matmuls run in bf16 (full PE rate), which only perturbs the
    already-normalized attention weights by ~1e-3 relative.
"""

from contextlib import ExitStack

import numpy as np

import concourse.bass as bass
import concourse.bacc as bacc
import concourse.tile as tile
from concourse import mybir
from concourse.bass_utils import run_bass_kernel_spmd
from concourse.masks import make_identity

B, L, H, Z = 32, 1024, 1024, 512
N_CORES = 8
B_PER = B // N_CORES
P = 128
INV_TEMP = 1.0 / float(np.sqrt(H))

F32 = mybir.dt.float32
BF16 = mybir.dt.bfloat16


def build_nc(b_per=B_PER, L=L, H=H, Z=Z):
    HB, ZB, LB = H // P, Z // P, L // P
    QS = min(512, L)  # matmul moving free-dim (one PSUM bank of fp32)
    nq = L // QS

    nc = bacc.Bacc()
    enc_d = nc.declare_dram_parameter("enc", [b_per, L, H], F32, isOutput=False)
    dec_d = nc.declare_dram_parameter("dec", [b_per, L, H], F32, isOutput=False)
    z_d = nc.declare_dram_parameter("z", [b_per, L, Z], F32, isOutput=False)
    madd_d = nc.declare_dram_parameter("madd", [b_per, L], F32, isOutput=False)
    g_d = nc.declare_dram_parameter("g", [H, H], F32, isOutput=False)
    wv_d = nc.declare_dram_parameter("wv", [Z, Z], F32, isOutput=False)
    out_d = nc.declare_dram_parameter("out", [b_per, L, Z], F32, isOutput=True)
    attn_d = nc.declare_dram_parameter("attn", [b_per, L, L], F32, isOutput=True)

    with tile.TileContext(nc) as tc, ExitStack() as ctx:
        const = ctx.enter_context(tc.tile_pool(name="const", bufs=1))
        actT_p = ctx.enter_context(tc.tile_pool(name="actT_p", bufs=1))
        kgt_p = ctx.enter_context(tc.tile_pool(name="kgt_p", bufs=1))
        pt_p = ctx.enter_context(tc.tile_pool(name="pt_p", bufs=1))
        ptb_p = ctx.enter_context(tc.tile_pool(name="ptb_p", bufs=1))
        v_p = ctx.enter_context(tc.tile_pool(name="v_p", bufs=1))
        madd_p = ctx.enter_context(tc.tile_pool(name="madd_p", bufs=2))
        loads = ctx.enter_context(tc.tile_pool(name="loads", bufs=3))
        stage = ctx.enter_context(tc.tile_pool(name="stage", bufs=2))
        stats = ctx.enter_context(tc.tile_pool(name="stats", bufs=8))
        psmm = ctx.enter_context(tc.tile_pool(name="psmm", bufs=4, space="PSUM"))
        pstp = ctx.enter_context(tc.tile_pool(name="pstp", bufs=4, space="PSUM"))

        ident = const.tile([P, P], F32)
        make_identity(nc, ident)

        # G[h', h] with h' on partitions (lhsT layout for KgT matmuls)
        g_t = const.tile([P, HB, H], F32)
        for i in range(HB):
            nc.gpsimd.dma_start(out=g_t[:, i, :], in_=g_d[i * P:(i + 1) * P, :])
        # W_v[z', z] with z' on partitions (rhs layout for V matmuls), in bf16
        wv_t = const.tile([P, ZB, Z], BF16)
        for i in range(ZB):
            wv_ld = loads.tile([P, Z], F32, tag="ld", name="wv_ld")
            nc.gpsimd.memset(wv_ld[0:1, 0:1], 0.0)
            nc.gpsimd.dma_start(out=wv_ld[:, :Z], in_=wv_d[i * P:(i + 1) * P, :])
            nc.any.tensor_copy(out=wv_t[:, i, :], in_=wv_ld[:, :Z])

        def probe(t):
            # A 1-element gpsimd write into a recycled slot absorbs the slot's
            # cross-engine WAR/WAW waits into the gpsimd sequencer clock, so
            # the DMA that follows needs only its own-lane wait (the HW DMA
            # pseudo-instruction supports at most 2 sync waits).
            nc.gpsimd.memset(t[0:1, 0:1], 0.0)

        def transpose_in(src_dram_2d, nrow_blocks, ncol_blocks, dst_tile):
            """dst[c_sub, cb, r] = src[r, c]; src rows partitioned into [P, width] loads."""
            for rb in range(nrow_blocks):
                ld = loads.tile([P, ncol_blocks * P], F32, tag="ld", name="ld")
                probe(ld)
                nc.gpsimd.dma_start(out=ld[:, :], in_=src_dram_2d[rb * P:(rb + 1) * P, :])
                for cb in range(ncol_blocks):
                    ps = pstp.tile([P, P], F32, tag="tp", name="tp_ps")
                    nc.tensor.transpose(ps, ld[:, cb * P:(cb + 1) * P], ident)
                    nc.any.tensor_copy(out=dst_tile[:, cb, rb * P:(rb + 1) * P], in_=ps)

        for b in range(b_per):
            # ---- decT[h', k] ----
            decT = actT_p.tile([P, HB, L], F32, tag="actT", name="decT")
            transpose_in(dec_d[b], LB, HB, decT)

            # ---- KgT[h, k] = sum_h' G[h', h] * decT[h', k] ----
            kgT = kgt_p.tile([P, HB, L], F32, tag="kgt", name="kgT")
            for hb in range(HB):
                for ks in range(nq):
                    ps = psmm.tile([P, QS], F32, tag="mm", name="mm_ps")
                    for hp in range(HB):
                        nc.tensor.matmul(
                            ps,
                            lhsT=g_t[:, hp, hb * P:(hb + 1) * P],
                            rhs=decT[:, hp, ks * QS:(ks + 1) * QS],
                            start=(hp == 0), stop=(hp == HB - 1),
                        )
                    nc.any.tensor_copy(out=kgT[:, hb, ks * QS:(ks + 1) * QS], in_=ps)

            # ---- encT[h, q] ----
            encT = actT_p.tile([P, HB, L], F32, tag="actT", name="encT")
            transpose_in(enc_d[b], LB, HB, encT)

            # mask: madd[b, q] broadcast over the 128 partitions
            madd_bc = madd_p.tile([P, L], F32, name="madd_bc")
            probe(madd_bc)
            nc.gpsimd.dma_start(
                out=madd_bc, in_=madd_d[b].unsqueeze(0).to_broadcast((P, L))
            )

            # ---- S^T[k, q] = sum_h KgT[h, k] * encT[h, q]  (+ mask over q) ----
            st = pt_p.tile([P, LB, L], F32, tag="pt", name="st")
            for kb in range(LB):
                for qs in range(nq):
                    ps = psmm.tile([P, QS], F32, tag="mm", name="mm_ps")
                    for hb in range(HB):
                        nc.tensor.matmul(
                            ps,
                            lhsT=kgT[:, hb, kb * P:(kb + 1) * P],
                            rhs=encT[:, hb, qs * QS:(qs + 1) * QS],
                            start=(hb == 0), stop=(hb == HB - 1),
                        )
                    nc.vector.tensor_add(
                        out=st[:, kb, qs * QS:(qs + 1) * QS],
                        in0=ps,
                        in1=madd_bc[:, qs * QS:(qs + 1) * QS],
                    )

            # ---- softmax over q (free axis): st -> P^T (f32, in place) and bf16 copy
            stb = ptb_p.tile([P, LB, L], BF16, tag="ptb", name="stb")
            for kb in range(LB):
                row = st[:, kb, :]
                negmax = stats.tile([P, 1], F32, tag="negmax", name="negmax")
                nc.vector.tensor_reduce(
                    out=negmax, in_=row, axis=mybir.AxisListType.X,
                    op=mybir.AluOpType.max, negate=True,
                )
                nc.scalar.mul(out=negmax, in_=negmax, mul=INV_TEMP)
                sums = stats.tile([P, 1], F32, tag="sums", name="sums")
                nc.scalar.activation(
                    out=row, in_=row, func=mybir.ActivationFunctionType.Exp,
                    bias=negmax, scale=INV_TEMP, accum_out=sums,
                )
                nc.vector.reciprocal(out=sums, in_=sums)
                nc.vector.tensor_scalar_mul(out=row, in0=row, scalar1=sums)
                nc.any.tensor_copy(out=stb[:, kb, :], in_=row)

            # ---- zT[z', k] (bf16) and V[k, z] = sum_z' zT[z', k] * W_v[z', z] (bf16)
            zT = kgt_p.tile([P, ZB, L], BF16, tag="kgt", name="zT")
            transpose_in(z_d[b], LB, ZB, zT)
            v_t = v_p.tile([P, LB, Z], BF16, tag="v", name="v_t")
            for kb in range(LB):
                ps = psmm.tile([P, Z], F32, tag="mm", name="mm_ps")
                for zb in range(ZB):
                    nc.tensor.matmul(
                        ps,
                        lhsT=zT[:, zb, kb * P:(kb + 1) * P],
                        rhs=wv_t[:, zb, :],
                        start=(zb == 0), stop=(zb == ZB - 1),
                    )
                nc.any.tensor_copy(out=v_t[:, kb, :], in_=ps)

            # ---- attn[b, q, k] = transpose(P^T)  (f32 path) ----
            for qb in range(LB):
                pq = stage.tile([P, L], F32, tag="pq", name="pq")
                for kb in range(LB):
                    ps = pstp.tile([P, P], F32, tag="tp", name="tp_ps")
                    nc.tensor.transpose(ps, st[:, kb, qb * P:(qb + 1) * P], ident)
                    nc.vector.tensor_copy(out=pq[:, kb * P:(kb + 1) * P], in_=ps)
                nc.sync.dma_start(out=attn_d[b, qb * P:(qb + 1) * P, :], in_=pq)

            # ---- out[b, q, z] = sum_k P^T[k, q] * V[k, z]  (bf16 matmul) ----
            for qb in range(LB):
                ps = psmm.tile([P, Z], F32, tag="mm", name="mm_ps")
                for kb in range(LB):
                    nc.tensor.matmul(
                        ps,
                        lhsT=stb[:, kb, qb * P:(qb + 1) * P],
                        rhs=v_t[:, kb, :],
                        start=(kb == 0), stop=(kb == LB - 1),
                    )
                outs = stage.tile([P, Z], F32, tag="outs", name="outs")
                nc.any.tensor_copy(out=outs, in_=ps)
                nc.sync.dma_start(out=out_d[b, qb * P:(qb + 1) * P, :], in_=outs)

    nc.finalize()
    return nc


_NC_CACHE = {}


def _get_nc(**kw):
    key = tuple(sorted(kw.items()))
    if key not in _NC_CACHE:
        _NC_CACHE[key] = build_nc(**kw)
    return _NC_CACHE[key]


def kernel(encoder_rnn_out, decoder_rnn_out, latent_z_seq, mask, W_q, W_k, W_v,
           **run_kw):
    enc = np.ascontiguousarray(encoder_rnn_out, dtype=np.float32)
    dec = np.ascontiguousarray(decoder_rnn_out, dtype=np.float32)
    z = np.ascontiguousarray(latent_z_seq, dtype=np.float32)
    G = (W_k.astype(np.float64) @ W_q.astype(np.float64).T).astype(np.float32)
    madd = np.where(np.asarray(mask) == 0, np.float32(-1e9), np.float32(0.0))
    wv = np.ascontiguousarray(W_v, dtype=np.float32)

    nc = _get_nc()
    in_maps = [
        {
            "enc": enc[c * B_PER:(c + 1) * B_PER],
            "dec": dec[c * B_PER:(c + 1) * B_PER],
            "z": z[c * B_PER:(c + 1) * B_PER],
            "madd": madd[c * B_PER:(c + 1) * B_PER],
            "g": G,
            "wv": wv,
        }
        for c in range(N_CORES)
    ]
    res = run_bass_kernel_spmd(nc, in_maps, core_ids=list(range(N_CORES)), **run_kw)
    out = np.concatenate([res.results[c]["out"] for c in range(N_CORES)], axis=0)
    attn = np.concatenate([res.results[c]["attn"] for c in range(N_CORES)], axis=0)
    if run_kw:
        kernel.last_results = res
    return out, attn


# revision 12
# speedup vs baseline: 1.1318x; 1.1318x over previous
"""Trainium2 Bass kernel for nn_Attention3 (cross-attention, softmax over query axis).

Math (per batch b):
    q = enc[b] @ W_q;  k = dec[b] @ W_k;  v = z[b] @ W_v
    S[q,k] = (q . k) / sqrt(H);  masked rows (mask[b,q]==0) -> -1e9
    attn = softmax over q axis;  out = attn-weighted sum of v

Kernel strategy (data-parallel over batch across 8 cores, 4 batches/core):
  - Host folds W_q/W_k into G = W_k @ W_q^T so the device computes
    S^T = dec @ G @ enc^T with two matmuls instead of three.
  - Everything on-device runs in the transposed S^T[k, q] layout so the
    softmax reduction over q is a free-axis (per-partition) reduce.
  - The score chain needs ~fp32 precision (softmax logits have std ~1e3; a
    plain bf16 matmul would corrupt the attention hard-max). Instead of the
    PE's 4-cycle/row fp32 mode, each fp32 operand is split into bf16
    hi + lo halves (x = hi + lo, exact to ~2^-18) and each score matmul runs
    as 3 bf16 passes (hi*hi + hi*lo + lo*hi) at 1 cycle/row - 25% faster
    with the dropped lo*lo term contributing ~2^-18 relative error.
  - enc/dec/G are split on the host; bf16 operands also unlock the XBAR
    DMA-transpose path, eliminating all PE input transposes.
  - Value/output matmuls run in plain bf16 (perturbs the normalized
    attention weights by ~1e-3 relative, well inside tolerance).
"""

from contextlib import ExitStack

import numpy as np
import ml_dtypes

import concourse.bass as bass
import concourse.bacc as bacc
import concourse.tile as tile
from concourse import mybir
from concourse.bass_utils import run_bass_kernel_spmd
from concourse.masks import make_identity

B, L, H, Z = 32, 1024, 1024, 512
N_CORES = 8
B_PER = B // N_CORES
P = 128
INV_TEMP = 1.0 / float(np.sqrt(H))

F32 = mybir.dt.float32
BF16 = mybir.dt.bfloat16


def build_nc(b_per=B_PER, L=L, H=H, Z=Z):
    HB, ZB, LB = H // P, Z // P, L // P
    QS = min(512, L)  # matmul moving free-dim (one PSUM bank of fp32)
    nq = L // QS

    nc = bacc.Bacc()
    enc_hi_d = nc.declare_dram_parameter("enc_hi", [b_per, L, H], BF16, isOutput=False)
    enc_lo_d = nc.declare_dram_parameter("enc_lo", [b_per, L, H], BF16, isOutput=False)
    dec_hi_d = nc.declare_dram_parameter("dec_hi", [b_per, L, H], BF16, isOutput=False)
    dec_lo_d = nc.declare_dram_parameter("dec_lo", [b_per, L, H], BF16, isOutput=False)
    z_d = nc.declare_dram_parameter("z", [b_per, L, Z], BF16, isOutput=False)
    madd_d = nc.declare_dram_parameter("madd", [b_per, L], F32, isOutput=False)
    g_hi_d = nc.declare_dram_parameter("g_hi", [H, H], BF16, isOutput=False)
    g_lo_d = nc.declare_dram_parameter("g_lo", [H, H], BF16, isOutput=False)
    wv_d = nc.declare_dram_parameter("wv", [Z, Z], BF16, isOutput=False)
    out_d = nc.declare_dram_parameter("out", [b_per, L, Z], F32, isOutput=True)
    attn_d = nc.declare_dram_parameter("attn", [b_per, L, L], F32, isOutput=True)

    with tile.TileContext(nc) as tc, ExitStack() as ctx:
        const = ctx.enter_context(tc.tile_pool(name="const", bufs=1))
        actT_p = ctx.enter_context(tc.tile_pool(name="actT_p", bufs=1))
        kgt_p = ctx.enter_context(tc.tile_pool(name="kgt_p", bufs=1))
        pt_p = ctx.enter_context(tc.tile_pool(name="pt_p", bufs=1))
        ptb_p = ctx.enter_context(tc.tile_pool(name="ptb_p", bufs=1))
        v_p = ctx.enter_context(tc.tile_pool(name="v_p", bufs=1))
        madd_p = ctx.enter_context(tc.tile_pool(name="madd_p", bufs=2))
        stage = ctx.enter_context(tc.tile_pool(name="stage", bufs=2))
        stats = ctx.enter_context(tc.tile_pool(name="stats", bufs=8))
        psmm = ctx.enter_context(tc.tile_pool(name="psmm", bufs=4, space="PSUM"))
        pstp = ctx.enter_context(tc.tile_pool(name="pstp", bufs=4, space="PSUM"))

        ident = const.tile([P, P], F32)
        make_identity(nc, ident)

        # G hi/lo [h', h] with h' on partitions (lhsT layout for KgT matmuls)
        g_t = const.tile([P, 2, HB, H], BF16)
        for half, gd in ((0, g_hi_d), (1, g_lo_d)):
            for i in range(HB):
                nc.sync.dma_start(out=g_t[:, half, i, :], in_=gd[i * P:(i + 1) * P, :])
        # W_v[z', z] with z' on partitions (rhs layout for V matmuls)
        wv_t = const.tile([P, ZB, Z], BF16)
        for i in range(ZB):
            nc.sync.dma_start(out=wv_t[:, i, :], in_=wv_d[i * P:(i + 1) * P, :])

        def transpose_load(src_hi, src_lo, ncol_blocks, dst_tile):
            """dst[c_sub, half, cb, r] = src_half[r, c] via XBAR DMA transpose."""
            srcs = ((0, src_hi),) if src_lo is None else ((0, src_hi), (1, src_lo))
            for half, src in srcs:
                for cb in range(ncol_blocks):
                    dst = dst_tile[:, half, cb, :] if src_lo is not None \
                        else dst_tile[:, cb, :]
                    nc.sync.dma_start_transpose(
                        out=dst, in_=src[:, cb * P:(cb + 1) * P])

        HILO = ((0, 0), (0, 1), (1, 0))  # (lhs_half, rhs_half) 3-pass split

        for b in range(b_per):
            # ---- decT hi/lo [h', k] ----
            decT = actT_p.tile([P, 2, HB, L], BF16, tag="actT", name="decT")
            transpose_load(dec_hi_d[b], dec_lo_d[b], HB, decT)

            # ---- KgT[h, k] = sum_h' G[h', h] * decT[h', k], split to hi/lo ----
            kgT = kgt_p.tile([P, 2, HB, L], BF16, tag="kgt", name="kgT")
            for hb in range(HB):
                for ks in range(nq):
                    ps = psmm.tile([P, QS], F32, tag="mm", name="mm_ps")
                    n3 = len(HILO)
                    for hp in range(HB):
                        for i3, (ga, da) in enumerate(HILO):
                            nc.tensor.matmul(
                                ps,
                                lhsT=g_t[:, ga, hp, hb * P:(hb + 1) * P],
                                rhs=decT[:, da, hp, ks * QS:(ks + 1) * QS],
                                start=(hp == 0 and i3 == 0),
                                stop=(hp == HB - 1 and i3 == n3 - 1),
                            )
                    hi_sl = kgT[:, 0, hb, ks * QS:(ks + 1) * QS]
                    nc.any.tensor_copy(out=hi_sl, in_=ps)
                    nc.vector.tensor_sub(
                        out=kgT[:, 1, hb, ks * QS:(ks + 1) * QS], in0=ps, in1=hi_sl)

            # ---- encT hi/lo [h, q] ----
            encT = actT_p.tile([P, 2, HB, L], BF16, tag="actT", name="encT")
            transpose_load(enc_hi_d[b], enc_lo_d[b], HB, encT)

            # mask: madd[b, q] broadcast over the 128 partitions
            madd_bc = madd_p.tile([P, L], F32, name="madd_bc")
            nc.gpsimd.dma_start(
                out=madd_bc, in_=madd_d[b].unsqueeze(0).to_broadcast((P, L))
            )

            # ---- S^T[k, q] = sum_h KgT[h, k] * encT[h, q]  (+ mask over q) ----
            st = pt_p.tile([P, LB, L], F32, tag="pt", name="st")
            for kb in range(LB):
                for qs in range(nq):
                    ps = psmm.tile([P, QS], F32, tag="mm", name="mm_ps")
                    n3 = len(HILO)
                    for hb in range(HB):
                        for i3, (ka, ea) in enumerate(HILO):
                            nc.tensor.matmul(
                                ps,
                                lhsT=kgT[:, ka, hb, kb * P:(kb + 1) * P],
                                rhs=encT[:, ea, hb, qs * QS:(qs + 1) * QS],
                                start=(hb == 0 and i3 == 0),
                                stop=(hb == HB - 1 and i3 == n3 - 1),
                            )
                    nc.vector.tensor_add(
                        out=st[:, kb, qs * QS:(qs + 1) * QS],
                        in0=ps,
                        in1=madd_bc[:, qs * QS:(qs + 1) * QS],
                    )

            # ---- softmax over q (free axis): st -> P^T (f32, in place) + bf16 copy
            stb = ptb_p.tile([P, LB, L], BF16, tag="ptb", name="stb")
            for kb in range(LB):
                row = st[:, kb, :]
                negmax = stats.tile([P, 1], F32, tag="negmax", name="negmax")
                nc.vector.tensor_reduce(
                    out=negmax, in_=row, axis=mybir.AxisListType.X,
                    op=mybir.AluOpType.max, negate=True,
                )
                nc.scalar.mul(out=negmax, in_=negmax, mul=INV_TEMP)
                sums = stats.tile([P, 1], F32, tag="sums", name="sums")
                nc.scalar.activation(
                    out=row, in_=row, func=mybir.ActivationFunctionType.Exp,
                    bias=negmax, scale=INV_TEMP, accum_out=sums,
                )
                nc.vector.reciprocal(out=sums, in_=sums)
                nc.vector.tensor_scalar_mul(out=row, in0=row, scalar1=sums)
                nc.any.tensor_copy(out=stb[:, kb, :], in_=row)

            # ---- zT[z', k] (bf16) and V[k, z] = sum_z' zT[z', k] * W_v[z', z] ----
            zT = kgt_p.tile([P, ZB, L], BF16, tag="kgt", name="zT")
            transpose_load(z_d[b], None, ZB, zT)
            v_t = v_p.tile([P, LB, Z], BF16, tag="v", name="v_t")
            for kb in range(LB):
                ps = psmm.tile([P, Z], F32, tag="mm", name="mm_ps")
                for zb in range(ZB):
                    nc.tensor.matmul(
                        ps,
                        lhsT=zT[:, zb, kb * P:(kb + 1) * P],
                        rhs=wv_t[:, zb, :],
                        start=(zb == 0), stop=(zb == ZB - 1),
                    )
                nc.any.tensor_copy(out=v_t[:, kb, :], in_=ps)

            # ---- attn[b, q, k] = transpose(P^T)  (f32 PE transposes) ----
            for qb in range(LB):
                pq = stage.tile([P, L], F32, tag="pq", name="pq")
                for kb in range(LB):
                    ps = pstp.tile([P, P], F32, tag="tp", name="tp_ps")
                    nc.tensor.transpose(ps, st[:, kb, qb * P:(qb + 1) * P], ident)
                    nc.vector.tensor_copy(out=pq[:, kb * P:(kb + 1) * P], in_=ps)
                nc.sync.dma_start(out=attn_d[b, qb * P:(qb + 1) * P, :], in_=pq)

            # ---- out[b, q, z] = sum_k P^T[k, q] * V[k, z]  (bf16 matmul) ----
            for qb in range(LB):
                ps = psmm.tile([P, Z], F32, tag="mm", name="mm_ps")
                for kb in range(LB):
                    nc.tensor.matmul(
                        ps,
                        lhsT=stb[:, kb, qb * P:(qb + 1) * P],
                        rhs=v_t[:, kb, :],
                        start=(kb == 0), stop=(kb == LB - 1),
                    )
                outs = stage.tile([P, Z], F32, tag="outs", name="outs")
                nc.any.tensor_copy(out=outs, in_=ps)
                nc.sync.dma_start(out=out_d[b, qb * P:(qb + 1) * P, :], in_=outs)

    nc.finalize()
    return nc


_NC_CACHE = {}


def _get_nc(**kw):
    key = tuple(sorted(kw.items()))
    if key not in _NC_CACHE:
        _NC_CACHE[key] = build_nc(**kw)
    return _NC_CACHE[key]


def _split_bf16(x):
    hi = x.astype(ml_dtypes.bfloat16)
    lo = (x - hi.astype(np.float32)).astype(ml_dtypes.bfloat16)
    return hi, lo


def kernel(encoder_rnn_out, decoder_rnn_out, latent_z_seq, mask, W_q, W_k, W_v,
           **run_kw):
    enc = np.ascontiguousarray(encoder_rnn_out, dtype=np.float32)
    dec = np.ascontiguousarray(decoder_rnn_out, dtype=np.float32)
    z = np.ascontiguousarray(latent_z_seq, dtype=np.float32)
    G = (W_k.astype(np.float64) @ W_q.astype(np.float64).T).astype(np.float32)
    madd = np.where(np.asarray(mask) == 0, np.float32(-1e9), np.float32(0.0))

    enc_hi, enc_lo = _split_bf16(enc)
    dec_hi, dec_lo = _split_bf16(dec)
    g_hi, g_lo = _split_bf16(G)
    z_bf = z.astype(ml_dtypes.bfloat16)
    wv_bf = np.asarray(W_v, dtype=np.float32).astype(ml_dtypes.bfloat16)

    nc = _get_nc()
    in_maps = [
        {
            "enc_hi": enc_hi[c * B_PER:(c + 1) * B_PER],
            "enc_lo": enc_lo[c * B_PER:(c + 1) * B_PER],
            "dec_hi": dec_hi[c * B_PER:(c + 1) * B_PER],
            "dec_lo": dec_lo[c * B_PER:(c + 1) * B_PER],
            "z": z_bf[c * B_PER:(c + 1) * B_PER],
            "madd": madd[c * B_PER:(c + 1) * B_PER],
            "g_hi": g_hi,
            "g_lo": g_lo,
            "wv": wv_bf,
        }
        for c in range(N_CORES)
    ]
    res = run_bass_kernel_spmd(nc, in_maps, core_ids=list(range(N_CORES)), **run_kw)
    out = np.concatenate([res.results[c]["out"] for c in range(N_CORES)], axis=0)
    attn = np.concatenate([res.results[c]["attn"] for c in range(N_CORES)], axis=0)
    if run_kw:
        kernel.last_results = res
    return out, attn
